# revision 26
# baseline (speedup 1.0000x reference)
"""Multi-head causal attention (B=2, T=2048, H=16, D=64, C=1024) on 8 trn2 cores.

Sharding: tensor-parallel over heads. Each core owns 2 heads (both batches):
  - computes Q^T/K^T/V^T for its heads over all 4096 tokens
  - causal attention in transposed orientation (S^T[k,q]) so no P transpose
  - partial output projection outT_partial[c, t] = Wo_slice^T @ O^T
Host sums the 8 partials (the "all-reduce"), adds bias, transposes back.

v3:
  - host pre-tiles x/weights into partition-major contiguous layouts; the
    first x block is DMAed per-ct so the first matmul starts early
  - QKV for batch 1 interleaved between batch-0 attention query blocks
  - scores double-buffered (2-bank tiles, exp per key tile) so the PE never
    waits on the scalar engine's exp
  - V ones-layout arranged so both heads' rowsums share partitions 0:64
    (single reciprocal per query block, read straight from PSUM)
  - bf16 partial output (halves write DMA; host sums in fp32)
"""

import sys

sys.path.insert(0, "/opt/trn_rl_repo")

import ml_dtypes
import numpy as np

import concourse.bacc as bacc
import concourse.mybir as mybir
import concourse.tile as tile
from concourse.bass_utils import run_bass_kernel_spmd

B, T, C = 2, 2048, 1024
H, D = 16, 64
NT = B * T  # 4096 flattened tokens
N_CORES = 8
HPC = H // N_CORES  # 2 heads per core
FPC = HPC * D  # 128 features per core
CT = C // 128  # 8 contraction tiles for projections
TBLK = 512  # token block
NTB = NT // TBLK  # 8 token blocks
QB = T // TBLK  # 4 query blocks per batch
KT = T // 128  # 16 key tiles per batch

F32 = mybir.dt.float32
BF16 = mybir.dt.bfloat16


def build_program():
    nc = bacc.Bacc("TRN2", target_bir_lowering=False, debug=False)

    # host pre-tiled layouts (partition-major, contiguous per DMA)
    xt_d = nc.declare_dram_parameter("xt", [NTB, 128, CT, TBLK], BF16, isOutput=False)
    wq_d = nc.declare_dram_parameter("wq", [128, CT, FPC], BF16, isOutput=False)
    wk_d = nc.declare_dram_parameter("wk", [128, CT, FPC], BF16, isOutput=False)
    wv_d = nc.declare_dram_parameter("wv", [128, CT, FPC], BF16, isOutput=False)
    wo_d = nc.declare_dram_parameter("wo", [FPC, C], BF16, isOutput=False)
    out_d = nc.declare_dram_parameter("outT", [NTB, 128, CT, TBLK], BF16, isOutput=True)

    with tile.TileContext(nc) as tc:
        with (
            tc.tile_pool(name="slabs", bufs=1) as slabs,
            tc.tile_pool(name="xtp", bufs=3) as xtp,
            tc.tile_pool(name="esp", bufs=12) as esp,
            tc.tile_pool(name="vtp", bufs=2) as vtp,
            tc.tile_pool(name="rinp", bufs=2) as rinp,
            tc.tile_pool(name="outp", bufs=2) as outp,
            tc.tile_pool(name="psS", bufs=2, space="PSUM") as psS,  # 2x2 banks
            tc.tile_pool(name="psO", bufs=1, space="PSUM") as psO,  # 2 banks
            tc.tile_pool(name="psA", bufs=2, space="PSUM") as psA,  # 2x1 banks
        ):
            # ---- persistent slabs
            qT = slabs.tile([128, NT], BF16, tag="qT")  # [2h*64d, t]
            kT = slabs.tile([128, NT], BF16, tag="kT")
            # V natural layout per ktile_global: [128k, (ones | V_h0 | ones | V_h1)]
            # PV stationary h = [:, ktg, 2h:2h+2, :] -> rowsum rows 0:64, O rows 64:128
            vN = slabs.tile([128, NTB * 4, 4, 64], BF16, tag="vN")
            oN = slabs.tile([128, NT], BF16, tag="oN")  # normalized O^T
            wq_s = slabs.tile([128, CT, FPC], BF16, tag="wq")
            wk_s = slabs.tile([128, CT, FPC], BF16, tag="wk")
            wv_s = slabs.tile([128, CT, FPC], BF16, tag="wv")
            wo_s = slabs.tile([128, C], BF16, tag="wo")  # [f, c]
            mtri2 = slabs.tile([128, 2, 128], BF16, tag="mtri2")  # 1 if j>=k else 0
            ident = slabs.tile([128, 128], BF16, tag="ident")

            # warm the PE HAM clock gate while the first DMAs land (a >=3.4us
            # busy burst moves the clock from 1.2 to 2.4 GHz before real work);
            # reads uninitialized SBUF on purpose - the result is never used
            scratch = slabs.tile([128, 128], BF16, tag="scratch")
            nc.gpsimd.memset(scratch[:], 0.0)
            warm = psA.tile([128, 128], F32, tag="ps", name="warm")
            for _ in range(56):
                nc.tensor.matmul(warm[:], scratch[:], scratch[:], start=True, stop=True)

            # ---- input loads first (the Sync queue issues in emission order;
            # the first Q matmul only needs wq + xt[0][:, 0, :])
            xt0 = xtp.tile([128, CT, TBLK], BF16, tag="xt")
            nc.sync.dma_start(wq_s[:], wq_d[:])
            nc.sync.dma_start(xt0[:, 0, :], xt_d[0, :, 0, :])
            nc.sync.dma_start(xt0[:, 1, :], xt_d[0, :, 1, :])
            nc.sync.dma_start(wk_s[:], wk_d[:])
            nc.sync.dma_start(xt0[:, 2:4, :], xt_d[0, :, 2:4, :])
            nc.sync.dma_start(wv_s[:], wv_d[:])
            nc.sync.dma_start(xt0[:, 4:, :], xt_d[0, :, 4:, :])
            nc.sync.dma_start(wo_s[:], wo_d[:])

            # ---- constants
            from concourse.masks import make_identity

            make_identity(nc, ident[:])
            mtri_f = slabs.tile([128, 128], F32, tag="mtri_f")
            nc.gpsimd.memset(mtri_f[:], 1.0)
            # keep 1.0 where (j - k) >= 0 else 0.0
            nc.gpsimd.affine_select(
                out=mtri_f[:],
                in_=mtri_f[:],
                compare_op=mybir.AluOpType.is_ge,
                fill=0.0,
                base=0,
                pattern=[[1, 128]],
                channel_multiplier=-1,
            )
            nc.vector.tensor_copy(mtri2[:, 0, :], mtri_f[:])
            nc.vector.tensor_copy(mtri2[:, 1, :], mtri_f[:])
            # ones columns of vN (constant for the whole run)
            nc.gpsimd.memset(vN[:, :, 0, :], 1.0)
            nc.gpsimd.memset(vN[:, :, 2, :], 1.0)

            # ---- QKV projections for one 512-token block
            def qkv_for_tb(tb, xt_t=None):
                if xt_t is None:
                    xt_t = xtp.tile([128, CT, TBLK], BF16, tag="xt")
                    nc.sync.dma_start(xt_t[:], xt_d[tb])
                for name, w_s, dstT in (("q", wq_s, qT), ("k", wk_s, kT)):
                    ps = psA.tile([128, TBLK], F32, tag="ps", name=f"ps_{name}_{tb}")
                    for ct in range(CT):
                        nc.tensor.matmul(
                            ps[:],
                            w_s[:, ct, :],
                            xt_t[:, ct, :],
                            start=(ct == 0),
                            stop=(ct == CT - 1),
                        )
                    nc.vector.tensor_copy(dstT[:, tb * TBLK : (tb + 1) * TBLK], ps[:])
                psv = psA.tile([128, TBLK], F32, tag="ps", name=f"ps_v_{tb}")
                for ct in range(CT):
                    nc.tensor.matmul(
                        psv[:],
                        wv_s[:, ct, :],
                        xt_t[:, ct, :],
                        start=(ct == 0),
                        stop=(ct == CT - 1),
                    )
                vt_t = vtp.tile([128, TBLK], BF16, tag="vt")
                nc.vector.tensor_copy(vt_t[:], psv[:])
                # transpose [128(d0|d1), 128k] -> [128k, (V_h0|V_h1)] on PE
                for sub in range(TBLK // 128):
                    ktg = tb * 4 + sub
                    tps = psA.tile([128, 128], BF16, tag="ps")
                    nc.tensor.transpose(
                        tps[:],
                        vt_t[:, sub * 128 : (sub + 1) * 128],
                        ident[:],
                    )
                    # strided dst: V_h0 -> slot 1, V_h1 -> slot 3
                    nc.vector.tensor_copy(
                        vN[:, ktg, 1:4:2, :],
                        tps[:].rearrange("p (a c) -> p a c", a=2),
                    )

            # out-projection for one (batch, qblock); evac="vector"|"scalar"|"mix"
            def outproj_qb(b, qb, evac="vector", chunked=False):
                t0 = b * T + qb * TBLK
                ot = outp.tile([128, CT, TBLK], BF16, tag="ot")
                for ct in range(CT):
                    opst = psA.tile([128, TBLK], F32, tag="ps")
                    nc.tensor.matmul(
                        opst[:],
                        wo_s[:, ct * 128 : (ct + 1) * 128],
                        oN[:, t0 : t0 + TBLK],
                        start=True,
                        stop=True,
                    )
                    if evac == "scalar" or (evac == "mix" and ct % 2 == 1):
                        nc.scalar.copy(ot[:, ct, :], opst[:])
                    else:
                        nc.vector.tensor_copy(ot[:, ct, :], opst[:])
                if chunked:
                    # chunked DMA so the store tail is short
                    for cc in range(0, CT, 2):
                        nc.sync.dma_start(
                            out_d[b * QB + qb, :, cc : cc + 2, :],
                            ot[:, cc : cc + 2, :],
                        )
                else:
                    nc.sync.dma_start(out_d[b * QB + qb], ot[:])

            # ---- attention for one (batch, qblock)
            def attention_qb(b, qb, outproj=True):
                t0 = b * T + qb * TBLK  # global token offset of this q block
                O_ps = psO.tile([128, HPC, TBLK], F32, tag="O", name=f"O_{b}_{qb}")
                nkt = (qb + 1) * 4
                for kt in range(nkt):
                    s = kt * 128 - qb * TBLK  # diag offset, >=0 on band
                    col0 = max(s, 0)
                    ktg = b * KT + kt
                    sT = psS.tile([128, HPC, TBLK], F32, tag="sT")
                    es = esp.tile([128, HPC, TBLK], BF16, tag="es")
                    for h in range(HPC):
                        hp = h * 64
                        nc.tensor.matmul(
                            sT[:, h, col0:TBLK],
                            kT[hp : hp + 64, b * T + kt * 128 : b * T + (kt + 1) * 128],
                            qT[hp : hp + 64, t0 + col0 : t0 + TBLK],
                            start=True,
                            stop=True,
                        )
                    nc.scalar.activation(
                        es[:, :, col0:TBLK],
                        sT[:, :, col0:TBLK],
                        mybir.ActivationFunctionType.Exp,
                        scale=0.125,
                    )
                    if s >= 0:  # diagonal tile: mask strict-lower triangle
                        nc.vector.tensor_mul(
                            es[:, :, col0 : col0 + 128],
                            es[:, :, col0 : col0 + 128],
                            mtri2[:],
                        )
                    for h in range(HPC):
                        nc.tensor.matmul(
                            O_ps[:, h, col0:TBLK],
                            vN[:, ktg, 2 * h : 2 * h + 2, :],
                            es[:, h, col0:TBLK],
                            start=(kt == 0),
                            stop=(kt == nkt - 1),
                        )
                # normalize: O rows 64:128 divided by rowsum rows 0:64 (both heads)
                rin = rinp.tile([64, HPC, TBLK], F32, tag="rin")
                nc.vector.reciprocal_approx_fast(rin[:], O_ps[0:64, :, :])
                nc.vector.tensor_mul(
                    oN[0:64, t0 : t0 + TBLK], O_ps[64:128, 0, :], rin[:, 0, :]
                )
                nc.vector.tensor_mul(
                    oN[64:128, t0 : t0 + TBLK], O_ps[64:128, 1, :], rin[:, 1, :]
                )
                if outproj:
                    outproj_qb(b, qb)

            # ---- schedule: QKV(b0) first, then b0 attention with QKV(b1)
            # blocks woven between query blocks, then b1 attention
            qkv_for_tb(0, xt_t=xt0)
            qkv_for_tb(1)
            attention_qb(0, 0)
            qkv_for_tb(2)
            attention_qb(0, 1)
            qkv_for_tb(3)
            attention_qb(0, 2)
            qkv_for_tb(4)
            attention_qb(0, 3)
            qkv_for_tb(5)
            attention_qb(1, 0)
            qkv_for_tb(6)
            attention_qb(1, 1)
            qkv_for_tb(7)
            attention_qb(1, 2, outproj=False)
            attention_qb(1, 3, outproj=False)
            # deferred projections: PE filler under the final exp stream;
            # their evacuation rides the then-idle scalar engine
            outproj_qb(1, 2, evac="scalar")
            outproj_qb(1, 3, evac="mix", chunked=True)

    nc.compile()
    return nc


_NC_CACHE = None


def get_program():
    global _NC_CACHE
    if _NC_CACHE is None:
        _NC_CACHE = build_program()
    return _NC_CACHE


def make_in_maps(x, Wq, Wk, Wv, Wo):
    bf = ml_dtypes.bfloat16
    xt = np.asarray(x, np.float32).reshape(NT, C)
    # x_pre[tb, p, ct, t] = x[tb*TBLK + t, ct*128 + p]
    x_pre = np.ascontiguousarray(
        xt.reshape(NTB, TBLK, CT, 128).transpose(0, 3, 2, 1)
    ).astype(bf)
    wq_b = np.asarray(Wq, np.float32).astype(bf)
    wk_b = np.asarray(Wk, np.float32).astype(bf)
    wv_b = np.asarray(Wv, np.float32).astype(bf)
    wo_b = np.asarray(Wo, np.float32).astype(bf)
    in_maps = []
    for cid in range(N_CORES):
        sl = slice(cid * FPC, (cid + 1) * FPC)

        def wpre(w):
            # w_pre[p, ct, f] = w[ct*128 + p, f]
            return np.ascontiguousarray(
                w[:, sl].reshape(CT, 128, FPC).transpose(1, 0, 2)
            )

        in_maps.append(
            {
                "xt": x_pre,
                "wq": wpre(wq_b),
                "wk": wpre(wk_b),
                "wv": wpre(wv_b),
                "wo": np.ascontiguousarray(wo_b[sl, :]),
            }
        )
    return in_maps


def kernel(x, Wq, Wk, Wv, Wo, bo, _trace=False, _tmpdir=None):
    x = np.asarray(x, dtype=np.float32)
    in_maps = make_in_maps(x, Wq, Wk, Wv, Wo)
    nc = get_program()
    res = run_bass_kernel_spmd(
        nc, in_maps, core_ids=list(range(N_CORES)), trace=_trace, tmpdir=_tmpdir
    )
    acc = res.results[0]["outT"].astype(np.float32)
    for i in range(1, N_CORES):
        acc = acc + res.results[i]["outT"].astype(np.float32)
    # acc[tb, p, ct, t] -> out[tb*TBLK + t, ct*128 + p]
    out = np.ascontiguousarray(acc.transpose(0, 3, 2, 1)).reshape(NT, C)
    out = out + np.asarray(bo, np.float32)[None, :]
    if _trace:
        kernel._last_results = res
    return out.reshape(B, T, C).astype(np.float32)


# revision 27
# speedup vs baseline: 1.0178x; 1.0178x over previous
"""Multi-head causal attention (B=2, T=2048, H=16, D=64, C=1024) on 8 trn2 cores.

Sharding: tensor-parallel over heads. Each core owns 2 heads (both batches):
  - computes Q^T/K^T/V^T for its heads over all 4096 tokens
  - causal attention in transposed orientation (S^T[k,q]) so no P transpose
  - partial output projection outT_partial[c, t] = Wo_slice^T @ O^T
Host sums the 8 partials (the "all-reduce"), adds bias, transposes back.

v3:
  - host pre-tiles x/weights into partition-major contiguous layouts; the
    first x block is DMAed per-ct so the first matmul starts early
  - QKV for batch 1 interleaved between batch-0 attention query blocks
  - scores double-buffered (2-bank tiles, exp per key tile) so the PE never
    waits on the scalar engine's exp
  - V ones-layout arranged so both heads' rowsums share partitions 0:64
    (single reciprocal per query block, read straight from PSUM)
  - bf16 partial output (halves write DMA; host sums in fp32)
"""

import sys

sys.path.insert(0, "/opt/trn_rl_repo")

import ml_dtypes
import numpy as np

import concourse.bacc as bacc
import concourse.mybir as mybir
import concourse.tile as tile
from concourse.bass_utils import run_bass_kernel_spmd

B, T, C = 2, 2048, 1024
H, D = 16, 64
NT = B * T  # 4096 flattened tokens
N_CORES = 8
HPC = H // N_CORES  # 2 heads per core
FPC = HPC * D  # 128 features per core
CT = C // 128  # 8 contraction tiles for projections
TBLK = 512  # token block
NTB = NT // TBLK  # 8 token blocks
QB = T // TBLK  # 4 query blocks per batch
KT = T // 128  # 16 key tiles per batch

F32 = mybir.dt.float32
BF16 = mybir.dt.bfloat16


def build_program():
    nc = bacc.Bacc("TRN2", target_bir_lowering=False, debug=False)

    # host pre-tiled layouts (partition-major, contiguous per DMA)
    xt_d = nc.declare_dram_parameter("xt", [NTB, 128, CT, TBLK], BF16, isOutput=False)
    wq_d = nc.declare_dram_parameter("wq", [128, CT, FPC], BF16, isOutput=False)
    wk_d = nc.declare_dram_parameter("wk", [128, CT, FPC], BF16, isOutput=False)
    wv_d = nc.declare_dram_parameter("wv", [128, CT, FPC], BF16, isOutput=False)
    wo_d = nc.declare_dram_parameter("wo", [FPC, C], BF16, isOutput=False)
    out_d = nc.declare_dram_parameter("outT", [NTB, 128, CT, TBLK], BF16, isOutput=True)

    with tile.TileContext(nc) as tc:
        with (
            tc.tile_pool(name="slabs", bufs=1) as slabs,
            tc.tile_pool(name="xtp", bufs=3) as xtp,
            tc.tile_pool(name="esp", bufs=12) as esp,
            tc.tile_pool(name="vtp", bufs=2) as vtp,
            tc.tile_pool(name="rinp", bufs=2) as rinp,
            tc.tile_pool(name="outp", bufs=2) as outp,
            tc.tile_pool(name="psS", bufs=2, space="PSUM") as psS,  # 2x2 banks
            tc.tile_pool(name="psO", bufs=1, space="PSUM") as psO,  # 2 banks
            tc.tile_pool(name="psA", bufs=2, space="PSUM") as psA,  # 2x1 banks
        ):
            # ---- persistent slabs
            qT = slabs.tile([128, NT], BF16, tag="qT")  # [2h*64d, t]
            kT = slabs.tile([128, NT], BF16, tag="kT")
            # V natural layout per ktile_global: [128k, (ones | V_h0 | ones | V_h1)]
            # PV stationary h = [:, ktg, 2h:2h+2, :] -> rowsum rows 0:64, O rows 64:128
            vN = slabs.tile([128, NTB * 4, 4, 64], BF16, tag="vN")
            oN = slabs.tile([128, NT], BF16, tag="oN")  # normalized O^T
            wq_s = slabs.tile([128, CT, FPC], BF16, tag="wq")
            wk_s = slabs.tile([128, CT, FPC], BF16, tag="wk")
            wv_s = slabs.tile([128, CT, FPC], BF16, tag="wv")
            wo_s = slabs.tile([128, C], BF16, tag="wo")  # [f, c]
            mtri2 = slabs.tile([128, 2, 128], BF16, tag="mtri2")  # 1 if j>=k else 0
            ident = slabs.tile([128, 128], BF16, tag="ident")

            # warm the PE HAM clock gate while the first DMAs land (a >=3.4us
            # busy burst moves the clock from 1.2 to 2.4 GHz before real work);
            # reads uninitialized SBUF on purpose - the result is never used
            scratch = slabs.tile([128, 128], BF16, tag="scratch")
            nc.gpsimd.memset(scratch[:], 0.0)
            warm = psA.tile([128, 128], F32, tag="ps", name="warm")
            for _ in range(36):
                nc.tensor.matmul(warm[:], scratch[:], scratch[:], start=True, stop=True)

            # ---- input loads first (the Sync queue issues in emission order;
            # the first Q matmul only needs wq + xt[0][:, 0, :])
            xt0 = xtp.tile([128, CT, TBLK], BF16, tag="xt")
            nc.sync.dma_start(wq_s[:], wq_d[:])
            nc.sync.dma_start(xt0[:, 0, :], xt_d[0, :, 0, :])
            nc.sync.dma_start(xt0[:, 1:, :], xt_d[0, :, 1:, :])
            nc.sync.dma_start(wk_s[:], wk_d[:])
            nc.sync.dma_start(wv_s[:], wv_d[:])
            nc.sync.dma_start(wo_s[:], wo_d[:])

            # ---- constants
            from concourse.masks import make_identity

            make_identity(nc, ident[:])
            mtri_f = slabs.tile([128, 128], F32, tag="mtri_f")
            nc.gpsimd.memset(mtri_f[:], 1.0)
            # keep 1.0 where (j - k) >= 0 else 0.0
            nc.gpsimd.affine_select(
                out=mtri_f[:],
                in_=mtri_f[:],
                compare_op=mybir.AluOpType.is_ge,
                fill=0.0,
                base=0,
                pattern=[[1, 128]],
                channel_multiplier=-1,
            )
            nc.vector.tensor_copy(mtri2[:, 0, :], mtri_f[:])
            nc.vector.tensor_copy(mtri2[:, 1, :], mtri_f[:])
            # ones columns of vN (constant for the whole run)
            nc.gpsimd.memset(vN[:, :, 0, :], 1.0)
            nc.gpsimd.memset(vN[:, :, 2, :], 1.0)

            # ---- QKV projections for one 512-token block
            def qkv_for_tb(tb, xt_t=None):
                if xt_t is None:
                    xt_t = xtp.tile([128, CT, TBLK], BF16, tag="xt")
                    nc.sync.dma_start(xt_t[:], xt_d[tb])
                for name, w_s, dstT in (("q", wq_s, qT), ("k", wk_s, kT)):
                    ps = psA.tile([128, TBLK], F32, tag="ps", name=f"ps_{name}_{tb}")
                    for ct in range(CT):
                        nc.tensor.matmul(
                            ps[:],
                            w_s[:, ct, :],
                            xt_t[:, ct, :],
                            start=(ct == 0),
                            stop=(ct == CT - 1),
                        )
                    nc.vector.tensor_copy(dstT[:, tb * TBLK : (tb + 1) * TBLK], ps[:])
                psv = psA.tile([128, TBLK], F32, tag="ps", name=f"ps_v_{tb}")
                for ct in range(CT):
                    nc.tensor.matmul(
                        psv[:],
                        wv_s[:, ct, :],
                        xt_t[:, ct, :],
                        start=(ct == 0),
                        stop=(ct == CT - 1),
                    )
                vt_t = vtp.tile([128, TBLK], BF16, tag="vt")
                nc.vector.tensor_copy(vt_t[:], psv[:])
                # transpose [128(d0|d1), 128k] -> [128k, (V_h0|V_h1)] on PE
                for sub in range(TBLK // 128):
                    ktg = tb * 4 + sub
                    tps = psA.tile([128, 128], BF16, tag="ps")
                    nc.tensor.transpose(
                        tps[:],
                        vt_t[:, sub * 128 : (sub + 1) * 128],
                        ident[:],
                    )
                    # strided dst: V_h0 -> slot 1, V_h1 -> slot 3
                    nc.vector.tensor_copy(
                        vN[:, ktg, 1:4:2, :],
                        tps[:].rearrange("p (a c) -> p a c", a=2),
                    )

            # out-projection for one (batch, qblock); evac="vector"|"scalar"|"mix"
            def outproj_qb(b, qb, evac="vector", chunked=False):
                t0 = b * T + qb * TBLK
                ot = outp.tile([128, CT, TBLK], BF16, tag="ot")
                for ct in range(CT):
                    opst = psA.tile([128, TBLK], F32, tag="ps")
                    nc.tensor.matmul(
                        opst[:],
                        wo_s[:, ct * 128 : (ct + 1) * 128],
                        oN[:, t0 : t0 + TBLK],
                        start=True,
                        stop=True,
                    )
                    if evac == "scalar" or (evac == "mix" and ct % 2 == 1):
                        nc.scalar.copy(ot[:, ct, :], opst[:])
                    else:
                        nc.vector.tensor_copy(ot[:, ct, :], opst[:])
                if chunked:
                    # chunked DMA so the store tail is short
                    for cc in range(0, CT, 2):
                        nc.sync.dma_start(
                            out_d[b * QB + qb, :, cc : cc + 2, :],
                            ot[:, cc : cc + 2, :],
                        )
                else:
                    nc.sync.dma_start(out_d[b * QB + qb], ot[:])

            # ---- attention for one (batch, qblock)
            def attention_qb(b, qb, outproj=True):
                t0 = b * T + qb * TBLK  # global token offset of this q block
                O_ps = psO.tile([128, HPC, TBLK], F32, tag="O", name=f"O_{b}_{qb}")
                nkt = (qb + 1) * 4
                for kt in range(nkt):
                    s = kt * 128 - qb * TBLK  # diag offset, >=0 on band
                    col0 = max(s, 0)
                    ktg = b * KT + kt
                    sT = psS.tile([128, HPC, TBLK], F32, tag="sT")
                    es = esp.tile([128, HPC, TBLK], BF16, tag="es")
                    for h in range(HPC):
                        hp = h * 64
                        nc.tensor.matmul(
                            sT[:, h, col0:TBLK],
                            kT[hp : hp + 64, b * T + kt * 128 : b * T + (kt + 1) * 128],
                            qT[hp : hp + 64, t0 + col0 : t0 + TBLK],
                            start=True,
                            stop=True,
                        )
                    nc.scalar.activation(
                        es[:, :, col0:TBLK],
                        sT[:, :, col0:TBLK],
                        mybir.ActivationFunctionType.Exp,
                        scale=0.125,
                    )
                    if s >= 0:  # diagonal tile: mask strict-lower triangle
                        nc.vector.tensor_mul(
                            es[:, :, col0 : col0 + 128],
                            es[:, :, col0 : col0 + 128],
                            mtri2[:],
                        )
                    for h in range(HPC):
                        nc.tensor.matmul(
                            O_ps[:, h, col0:TBLK],
                            vN[:, ktg, 2 * h : 2 * h + 2, :],
                            es[:, h, col0:TBLK],
                            start=(kt == 0),
                            stop=(kt == nkt - 1),
                        )
                # normalize: O rows 64:128 divided by rowsum rows 0:64 (both heads)
                rin = rinp.tile([64, HPC, TBLK], F32, tag="rin")
                nc.vector.reciprocal_approx_fast(rin[:], O_ps[0:64, :, :])
                nc.vector.tensor_mul(
                    oN[0:64, t0 : t0 + TBLK], O_ps[64:128, 0, :], rin[:, 0, :]
                )
                nc.vector.tensor_mul(
                    oN[64:128, t0 : t0 + TBLK], O_ps[64:128, 1, :], rin[:, 1, :]
                )
                if outproj:
                    outproj_qb(b, qb)

            # ---- schedule: QKV(b0) first, then b0 attention with QKV(b1)
            # blocks woven between query blocks, then b1 attention
            qkv_for_tb(0, xt_t=xt0)
            qkv_for_tb(1)
            attention_qb(0, 0)
            qkv_for_tb(2)
            attention_qb(0, 1)
            qkv_for_tb(3)
            attention_qb(0, 2)
            qkv_for_tb(4)
            attention_qb(0, 3)
            qkv_for_tb(5)
            attention_qb(1, 0)
            qkv_for_tb(6)
            attention_qb(1, 1)
            qkv_for_tb(7)
            attention_qb(1, 2, outproj=False)
            attention_qb(1, 3, outproj=False)
            # deferred projections: PE filler under the final exp stream;
            # their evacuation rides the then-idle scalar engine
            outproj_qb(1, 2, evac="scalar")
            outproj_qb(1, 3, evac="mix", chunked=True)

    nc.compile()
    return nc


_NC_CACHE = None


def get_program():
    global _NC_CACHE
    if _NC_CACHE is None:
        _NC_CACHE = build_program()
    return _NC_CACHE


def make_in_maps(x, Wq, Wk, Wv, Wo):
    bf = ml_dtypes.bfloat16
    xt = np.asarray(x, np.float32).reshape(NT, C)
    # x_pre[tb, p, ct, t] = x[tb*TBLK + t, ct*128 + p]
    x_pre = np.ascontiguousarray(
        xt.reshape(NTB, TBLK, CT, 128).transpose(0, 3, 2, 1)
    ).astype(bf)
    wq_b = np.asarray(Wq, np.float32).astype(bf)
    wk_b = np.asarray(Wk, np.float32).astype(bf)
    wv_b = np.asarray(Wv, np.float32).astype(bf)
    wo_b = np.asarray(Wo, np.float32).astype(bf)
    in_maps = []
    for cid in range(N_CORES):
        sl = slice(cid * FPC, (cid + 1) * FPC)

        def wpre(w):
            # w_pre[p, ct, f] = w[ct*128 + p, f]
            return np.ascontiguousarray(
                w[:, sl].reshape(CT, 128, FPC).transpose(1, 0, 2)
            )

        in_maps.append(
            {
                "xt": x_pre,
                "wq": wpre(wq_b),
                "wk": wpre(wk_b),
                "wv": wpre(wv_b),
                "wo": np.ascontiguousarray(wo_b[sl, :]),
            }
        )
    return in_maps


def kernel(x, Wq, Wk, Wv, Wo, bo, _trace=False, _tmpdir=None):
    x = np.asarray(x, dtype=np.float32)
    in_maps = make_in_maps(x, Wq, Wk, Wv, Wo)
    nc = get_program()
    res = run_bass_kernel_spmd(
        nc, in_maps, core_ids=list(range(N_CORES)), trace=_trace, tmpdir=_tmpdir
    )
    acc = res.results[0]["outT"].astype(np.float32)
    for i in range(1, N_CORES):
        acc = acc + res.results[i]["outT"].astype(np.float32)
    # acc[tb, p, ct, t] -> out[tb*TBLK + t, ct*128 + p]
    out = np.ascontiguousarray(acc.transpose(0, 3, 2, 1)).reshape(NT, C)
    out = out + np.asarray(bo, np.float32)[None, :]
    if _trace:
        kernel._last_results = res
    return out.reshape(B, T, C).astype(np.float32)


# revision 28
# speedup vs baseline: 1.0266x; 1.0086x over previous
"""Multi-head causal attention (B=2, T=2048, H=16, D=64, C=1024) on 8 trn2 cores.

Sharding: tensor-parallel over heads. Each core owns 2 heads (both batches):
  - computes Q^T/K^T/V^T for its heads over all 4096 tokens
  - causal attention in transposed orientation (S^T[k,q]) so no P transpose
  - partial output projection outT_partial[c, t] = Wo_slice^T @ O^T
Host sums the 8 partials (the "all-reduce"), adds bias, transposes back.

v3:
  - host pre-tiles x/weights into partition-major contiguous layouts; the
    first x block is DMAed per-ct so the first matmul starts early
  - QKV for batch 1 interleaved between batch-0 attention query blocks
  - scores double-buffered (2-bank tiles, exp per key tile) so the PE never
    waits on the scalar engine's exp
  - V ones-layout arranged so both heads' rowsums share partitions 0:64
    (single reciprocal per query block, read straight from PSUM)
  - bf16 partial output (halves write DMA; host sums in fp32)
"""

import sys

sys.path.insert(0, "/opt/trn_rl_repo")

import ml_dtypes
import numpy as np

import concourse.bacc as bacc
import concourse.mybir as mybir
import concourse.tile as tile
from concourse.bass_utils import run_bass_kernel_spmd

B, T, C = 2, 2048, 1024
H, D = 16, 64
NT = B * T  # 4096 flattened tokens
N_CORES = 8
HPC = H // N_CORES  # 2 heads per core
FPC = HPC * D  # 128 features per core
CT = C // 128  # 8 contraction tiles for projections
TBLK = 512  # token block
NTB = NT // TBLK  # 8 token blocks
QB = T // TBLK  # 4 query blocks per batch
KT = T // 128  # 16 key tiles per batch

F32 = mybir.dt.float32
BF16 = mybir.dt.bfloat16


def build_program():
    nc = bacc.Bacc("TRN2", target_bir_lowering=False, debug=False)

    # host pre-tiled layouts (partition-major, contiguous per DMA)
    xt_d = nc.declare_dram_parameter("xt", [NTB, 128, CT, TBLK], BF16, isOutput=False)
    wq_d = nc.declare_dram_parameter("wq", [128, CT, FPC], BF16, isOutput=False)
    wk_d = nc.declare_dram_parameter("wk", [128, CT, FPC], BF16, isOutput=False)
    wv_d = nc.declare_dram_parameter("wv", [128, CT, FPC], BF16, isOutput=False)
    wo_d = nc.declare_dram_parameter("wo", [FPC, C], BF16, isOutput=False)
    out_d = nc.declare_dram_parameter("outT", [NTB, 128, CT, TBLK], BF16, isOutput=True)

    with tile.TileContext(nc) as tc:
        with (
            tc.tile_pool(name="slabs", bufs=1) as slabs,
            tc.tile_pool(name="xtp", bufs=3) as xtp,
            tc.tile_pool(name="esp", bufs=12) as esp,
            tc.tile_pool(name="vtp", bufs=2) as vtp,
            tc.tile_pool(name="rinp", bufs=2) as rinp,
            tc.tile_pool(name="outp", bufs=2) as outp,
            tc.tile_pool(name="psS", bufs=2, space="PSUM") as psS,  # 2x2 banks
            tc.tile_pool(name="psO", bufs=1, space="PSUM") as psO,  # 2 banks
            tc.tile_pool(name="psA", bufs=2, space="PSUM") as psA,  # 2x1 banks
        ):
            # ---- persistent slabs
            qT = slabs.tile([128, NT], BF16, tag="qT")  # [2h*64d, t]
            kT = slabs.tile([128, NT], BF16, tag="kT")
            # V natural layout per ktile_global: [128k, (ones | V_h0 | ones | V_h1)]
            # PV stationary h = [:, ktg, 2h:2h+2, :] -> rowsum rows 0:64, O rows 64:128
            vN = slabs.tile([128, NTB * 4, 4, 64], BF16, tag="vN")
            oN = slabs.tile([128, NT], BF16, tag="oN")  # normalized O^T
            wq_s = slabs.tile([128, CT, FPC], BF16, tag="wq")
            wk_s = slabs.tile([128, CT, FPC], BF16, tag="wk")
            wv_s = slabs.tile([128, CT, FPC], BF16, tag="wv")
            wo_s = slabs.tile([128, C], BF16, tag="wo")  # [f, c]
            mtri2 = slabs.tile([128, 2, 128], BF16, tag="mtri2")  # 1 if j>=k else 0
            ident = slabs.tile([128, 128], BF16, tag="ident")

            # warm the PE HAM clock gate while the first DMAs land (a >=3.4us
            # busy burst moves the clock from 1.2 to 2.4 GHz before real work);
            # reads uninitialized SBUF on purpose - the result is never used
            scratch = slabs.tile([128, 128], BF16, tag="scratch")
            nc.gpsimd.memset(scratch[:], 0.0)
            warm = psA.tile([128, 128], F32, tag="ps", name="warm")
            for _ in range(36):
                nc.tensor.matmul(warm[:], scratch[:], scratch[:], start=True, stop=True)

            # ---- input loads first (the Sync queue issues in emission order;
            # the first Q matmul only needs wq + xt[0][:, 0, :])
            xt0 = xtp.tile([128, CT, TBLK], BF16, tag="xt")
            nc.sync.dma_start(wq_s[:], wq_d[:])
            nc.sync.dma_start(xt0[:, 0, :], xt_d[0, :, 0, :])
            nc.sync.dma_start(xt0[:, 1:, :], xt_d[0, :, 1:, :])
            nc.sync.dma_start(wk_s[:], wk_d[:])
            nc.sync.dma_start(wv_s[:], wv_d[:])
            nc.sync.dma_start(wo_s[:], wo_d[:])

            # ---- constants
            from concourse.masks import make_identity

            make_identity(nc, ident[:])
            mtri_f = slabs.tile([128, 128], F32, tag="mtri_f")
            nc.gpsimd.memset(mtri_f[:], 1.0)
            # keep 1.0 where (j - k) >= 0 else 0.0
            nc.gpsimd.affine_select(
                out=mtri_f[:],
                in_=mtri_f[:],
                compare_op=mybir.AluOpType.is_ge,
                fill=0.0,
                base=0,
                pattern=[[1, 128]],
                channel_multiplier=-1,
            )
            nc.vector.tensor_copy(mtri2[:, 0, :], mtri_f[:])
            nc.vector.tensor_copy(mtri2[:, 1, :], mtri_f[:])
            # ones columns of vN (constant for the whole run)
            nc.gpsimd.memset(vN[:, :, 0, :], 1.0)
            nc.gpsimd.memset(vN[:, :, 2, :], 1.0)

            # ---- QKV projections for one 512-token block
            def qkv_for_tb(tb, xt_t=None):
                if xt_t is None:
                    xt_t = xtp.tile([128, CT, TBLK], BF16, tag="xt")
                    nc.sync.dma_start(xt_t[:], xt_d[tb])
                for name, w_s, dstT in (("q", wq_s, qT), ("k", wk_s, kT)):
                    ps = psA.tile([128, TBLK], F32, tag="ps", name=f"ps_{name}_{tb}")
                    for ct in range(CT):
                        nc.tensor.matmul(
                            ps[:],
                            w_s[:, ct, :],
                            xt_t[:, ct, :],
                            start=(ct == 0),
                            stop=(ct == CT - 1),
                        )
                    nc.vector.tensor_copy(dstT[:, tb * TBLK : (tb + 1) * TBLK], ps[:])
                psv = psA.tile([128, TBLK], F32, tag="ps", name=f"ps_v_{tb}")
                for ct in range(CT):
                    nc.tensor.matmul(
                        psv[:],
                        wv_s[:, ct, :],
                        xt_t[:, ct, :],
                        start=(ct == 0),
                        stop=(ct == CT - 1),
                    )
                vt_t = vtp.tile([128, TBLK], BF16, tag="vt")
                nc.vector.tensor_copy(vt_t[:], psv[:])
                # transpose [128(d0|d1), 128k] -> [128k, (V_h0|V_h1)] on PE
                for sub in range(TBLK // 128):
                    ktg = tb * 4 + sub
                    tps = psA.tile([128, 128], BF16, tag="ps")
                    nc.tensor.transpose(
                        tps[:],
                        vt_t[:, sub * 128 : (sub + 1) * 128],
                        ident[:],
                    )
                    # strided dst: V_h0 -> slot 1, V_h1 -> slot 3
                    nc.vector.tensor_copy(
                        vN[:, ktg, 1:4:2, :],
                        tps[:].rearrange("p (a c) -> p a c", a=2),
                    )

            # out-projection for one (batch, qblock); evac="vector"|"scalar"|"mix"
            def outproj_qb(b, qb, evac="vector", chunked=False):
                t0 = b * T + qb * TBLK
                ot = outp.tile([128, CT, TBLK], BF16, tag="ot")
                for ct in range(CT):
                    opst = psA.tile([128, TBLK], F32, tag="ps")
                    nc.tensor.matmul(
                        opst[:],
                        wo_s[:, ct * 128 : (ct + 1) * 128],
                        oN[:, t0 : t0 + TBLK],
                        start=True,
                        stop=True,
                    )
                    if evac == "scalar" or (evac == "mix" and ct % 2 == 1):
                        nc.scalar.copy(ot[:, ct, :], opst[:])
                    else:
                        nc.vector.tensor_copy(ot[:, ct, :], opst[:])
                if chunked:
                    # chunked DMA so the store tail is short
                    for cc in range(0, CT, 2):
                        nc.sync.dma_start(
                            out_d[b * QB + qb, :, cc : cc + 2, :],
                            ot[:, cc : cc + 2, :],
                        )
                else:
                    nc.sync.dma_start(out_d[b * QB + qb], ot[:])

            # ---- attention for one (batch, qblock)
            def attention_qb(b, qb, outproj=True):
                t0 = b * T + qb * TBLK  # global token offset of this q block
                O_ps = psO.tile([128, HPC, TBLK], F32, tag="O", name=f"O_{b}_{qb}")
                nkt = (qb + 1) * 4
                for kt in range(nkt):
                    s = kt * 128 - qb * TBLK  # diag offset, >=0 on band
                    col0 = max(s, 0)
                    ktg = b * KT + kt
                    sT = psS.tile([128, HPC, TBLK], F32, tag="sT")
                    es = esp.tile([128, HPC, TBLK], BF16, tag="es")
                    for h in range(HPC):
                        hp = h * 64
                        nc.tensor.matmul(
                            sT[:, h, col0:TBLK],
                            kT[hp : hp + 64, b * T + kt * 128 : b * T + (kt + 1) * 128],
                            qT[hp : hp + 64, t0 + col0 : t0 + TBLK],
                            start=True,
                            stop=True,
                        )
                    nc.scalar.activation(
                        es[:, :, col0:TBLK],
                        sT[:, :, col0:TBLK],
                        mybir.ActivationFunctionType.Exp,
                        scale=0.125,
                    )
                    if s >= 0:  # diagonal tile: mask strict-lower triangle
                        nc.vector.tensor_mul(
                            es[:, :, col0 : col0 + 128],
                            es[:, :, col0 : col0 + 128],
                            mtri2[:],
                        )
                    for h in range(HPC):
                        nc.tensor.matmul(
                            O_ps[:, h, col0:TBLK],
                            vN[:, ktg, 2 * h : 2 * h + 2, :],
                            es[:, h, col0:TBLK],
                            start=(kt == 0),
                            stop=(kt == nkt - 1),
                        )
                # normalize: O rows 64:128 divided by rowsum rows 0:64 (both heads)
                rin = rinp.tile([64, HPC, TBLK], F32, tag="rin")
                nc.vector.reciprocal_approx_fast(rin[:], O_ps[0:64, :, :])
                nc.vector.tensor_mul(
                    oN[0:64, t0 : t0 + TBLK], O_ps[64:128, 0, :], rin[:, 0, :]
                )
                nc.vector.tensor_mul(
                    oN[64:128, t0 : t0 + TBLK], O_ps[64:128, 1, :], rin[:, 1, :]
                )
                if outproj:
                    outproj_qb(b, qb)

            # ---- schedule: QKV(b0) first, then b0 attention with QKV(b1)
            # blocks woven between query blocks, then b1 attention
            qkv_for_tb(0, xt_t=xt0)
            qkv_for_tb(1)
            attention_qb(0, 0)
            qkv_for_tb(2)
            attention_qb(0, 1)
            qkv_for_tb(3)
            attention_qb(0, 2)
            qkv_for_tb(4)
            attention_qb(0, 3)
            qkv_for_tb(5)
            attention_qb(1, 0)
            qkv_for_tb(6)
            attention_qb(1, 1)
            qkv_for_tb(7)
            attention_qb(1, 2)
            attention_qb(1, 3, outproj=False)
            outproj_qb(1, 3, evac="mix", chunked=True)

    nc.compile()
    return nc


_NC_CACHE = None


def get_program():
    global _NC_CACHE
    if _NC_CACHE is None:
        _NC_CACHE = build_program()
    return _NC_CACHE


def make_in_maps(x, Wq, Wk, Wv, Wo):
    bf = ml_dtypes.bfloat16
    xt = np.asarray(x, np.float32).reshape(NT, C)
    # x_pre[tb, p, ct, t] = x[tb*TBLK + t, ct*128 + p]
    x_pre = np.ascontiguousarray(
        xt.reshape(NTB, TBLK, CT, 128).transpose(0, 3, 2, 1)
    ).astype(bf)
    wq_b = np.asarray(Wq, np.float32).astype(bf)
    wk_b = np.asarray(Wk, np.float32).astype(bf)
    wv_b = np.asarray(Wv, np.float32).astype(bf)
    wo_b = np.asarray(Wo, np.float32).astype(bf)
    in_maps = []
    for cid in range(N_CORES):
        sl = slice(cid * FPC, (cid + 1) * FPC)

        def wpre(w):
            # w_pre[p, ct, f] = w[ct*128 + p, f]
            return np.ascontiguousarray(
                w[:, sl].reshape(CT, 128, FPC).transpose(1, 0, 2)
            )

        in_maps.append(
            {
                "xt": x_pre,
                "wq": wpre(wq_b),
                "wk": wpre(wk_b),
                "wv": wpre(wv_b),
                "wo": np.ascontiguousarray(wo_b[sl, :]),
            }
        )
    return in_maps


def kernel(x, Wq, Wk, Wv, Wo, bo, _trace=False, _tmpdir=None):
    x = np.asarray(x, dtype=np.float32)
    in_maps = make_in_maps(x, Wq, Wk, Wv, Wo)
    nc = get_program()
    res = run_bass_kernel_spmd(
        nc, in_maps, core_ids=list(range(N_CORES)), trace=_trace, tmpdir=_tmpdir
    )
    acc = res.results[0]["outT"].astype(np.float32)
    for i in range(1, N_CORES):
        acc = acc + res.results[i]["outT"].astype(np.float32)
    # acc[tb, p, ct, t] -> out[tb*TBLK + t, ct*128 + p]
    out = np.ascontiguousarray(acc.transpose(0, 3, 2, 1)).reshape(NT, C)
    out = out + np.asarray(bo, np.float32)[None, :]
    if _trace:
        kernel._last_results = res
    return out.reshape(B, T, C).astype(np.float32)


# revision 31
# speedup vs baseline: 1.0298x; 1.0032x over previous
"""Multi-head causal attention (B=2, T=2048, H=16, D=64, C=1024) on 8 trn2 cores.

Sharding: tensor-parallel over heads. Each core owns 2 heads (both batches):
  - computes Q^T/K^T/V^T for its heads over all 4096 tokens
  - causal attention in transposed orientation (S^T[k,q]) so no P transpose
  - partial output projection outT_partial[c, t] = Wo_slice^T @ O^T
Host sums the 8 partials (the "all-reduce"), adds bias, transposes back.

v3:
  - host pre-tiles x/weights into partition-major contiguous layouts; the
    first x block is DMAed per-ct so the first matmul starts early
  - QKV for batch 1 interleaved between batch-0 attention query blocks
  - scores double-buffered (2-bank tiles, exp per key tile) so the PE never
    waits on the scalar engine's exp
  - V ones-layout arranged so both heads' rowsums share partitions 0:64
    (single reciprocal per query block, read straight from PSUM)
  - bf16 partial output (halves write DMA; host sums in fp32)
"""

import sys

sys.path.insert(0, "/opt/trn_rl_repo")

import ml_dtypes
import numpy as np

import concourse.bacc as bacc
import concourse.mybir as mybir
import concourse.tile as tile
from concourse.bass_utils import run_bass_kernel_spmd

B, T, C = 2, 2048, 1024
H, D = 16, 64
NT = B * T  # 4096 flattened tokens
N_CORES = 8
HPC = H // N_CORES  # 2 heads per core
FPC = HPC * D  # 128 features per core
CT = C // 128  # 8 contraction tiles for projections
TBLK = 512  # token block
NTB = NT // TBLK  # 8 token blocks
QB = T // TBLK  # 4 query blocks per batch
KT = T // 128  # 16 key tiles per batch

F32 = mybir.dt.float32
BF16 = mybir.dt.bfloat16


def build_program():
    nc = bacc.Bacc("TRN2", target_bir_lowering=False, debug=False)

    # host pre-tiled layouts (partition-major, contiguous per DMA)
    xt_d = nc.declare_dram_parameter("xt", [NTB, 128, CT, TBLK], BF16, isOutput=False)
    wq_d = nc.declare_dram_parameter("wq", [128, CT, FPC], BF16, isOutput=False)
    wk_d = nc.declare_dram_parameter("wk", [128, CT, FPC], BF16, isOutput=False)
    wv_d = nc.declare_dram_parameter("wv", [128, CT, FPC], BF16, isOutput=False)
    wo_d = nc.declare_dram_parameter("wo", [FPC, C], BF16, isOutput=False)
    out_d = nc.declare_dram_parameter("outT", [NTB, 128, CT, TBLK], BF16, isOutput=True)

    with tile.TileContext(nc) as tc:
        with (
            tc.tile_pool(name="slabs", bufs=1) as slabs,
            tc.tile_pool(name="xtp", bufs=3) as xtp,
            tc.tile_pool(name="esp", bufs=12) as esp,
            tc.tile_pool(name="vtp", bufs=2) as vtp,
            tc.tile_pool(name="rinp", bufs=2) as rinp,
            tc.tile_pool(name="outp", bufs=2) as outp,
            tc.tile_pool(name="psS", bufs=2, space="PSUM") as psS,  # 2x2 banks
            tc.tile_pool(name="psO", bufs=1, space="PSUM") as psO,  # 2 banks
            tc.tile_pool(name="psA", bufs=2, space="PSUM") as psA,  # 2x1 banks
        ):
            # ---- persistent slabs
            qT = slabs.tile([128, NT], BF16, tag="qT")  # [2h*64d, t]
            kT = slabs.tile([128, NT], BF16, tag="kT")
            # V natural layout per ktile_global: [128k, (ones | V_h0 | ones | V_h1)]
            # PV stationary h = [:, ktg, 2h:2h+2, :] -> rowsum rows 0:64, O rows 64:128
            vN = slabs.tile([128, NTB * 4, 4, 64], BF16, tag="vN")
            oN = slabs.tile([128, NT], BF16, tag="oN")  # normalized O^T
            wq_s = slabs.tile([128, CT, FPC], BF16, tag="wq")
            wk_s = slabs.tile([128, CT, FPC], BF16, tag="wk")
            wv_s = slabs.tile([128, CT, FPC], BF16, tag="wv")
            wo_s = slabs.tile([128, C], BF16, tag="wo")  # [f, c]
            mtri2 = slabs.tile([128, 2, 128], BF16, tag="mtri2")  # 1 if j>=k else 0
            ident = slabs.tile([128, 128], BF16, tag="ident")

            # warm the PE HAM clock gate while the first DMAs land (a >=3.4us
            # busy burst moves the clock from 1.2 to 2.4 GHz before real work);
            # reads uninitialized SBUF on purpose - the result is never used
            scratch = slabs.tile([128, 128], BF16, tag="scratch")
            nc.gpsimd.memset(scratch[:], 0.0)
            warm = psA.tile([128, 128], F32, tag="ps", name="warm")
            for _ in range(36):
                nc.tensor.matmul(warm[:], scratch[:], scratch[:], start=True, stop=True)

            # ---- input loads first (the Sync queue issues in emission order;
            # the first Q matmul only needs wq + xt[0][:, 0, :])
            xt0 = xtp.tile([128, CT, TBLK], BF16, tag="xt")
            nc.sync.dma_start(wq_s[:], wq_d[:])
            nc.sync.dma_start(xt0[:, 0, :], xt_d[0, :, 0, :])
            nc.sync.dma_start(xt0[:, 1:4, :], xt_d[0, :, 1:4, :])
            nc.sync.dma_start(xt0[:, 4:, :], xt_d[0, :, 4:, :])
            nc.sync.dma_start(wk_s[:], wk_d[:])
            nc.sync.dma_start(wv_s[:], wv_d[:])
            nc.sync.dma_start(wo_s[:], wo_d[:])

            # ---- constants
            from concourse.masks import make_identity

            make_identity(nc, ident[:])
            mtri_f = slabs.tile([128, 128], F32, tag="mtri_f")
            nc.gpsimd.memset(mtri_f[:], 1.0)
            # keep 1.0 where (j - k) >= 0 else 0.0
            nc.gpsimd.affine_select(
                out=mtri_f[:],
                in_=mtri_f[:],
                compare_op=mybir.AluOpType.is_ge,
                fill=0.0,
                base=0,
                pattern=[[1, 128]],
                channel_multiplier=-1,
            )
            nc.vector.tensor_copy(mtri2[:, 0, :], mtri_f[:])
            nc.vector.tensor_copy(mtri2[:, 1, :], mtri_f[:])
            # ones columns of vN (constant for the whole run)
            nc.gpsimd.memset(vN[:, :, 0, :], 1.0)
            nc.gpsimd.memset(vN[:, :, 2, :], 1.0)

            # ---- QKV projections for one 512-token block
            def qkv_for_tb(tb, xt_t=None):
                if xt_t is None:
                    xt_t = xtp.tile([128, CT, TBLK], BF16, tag="xt")
                    nc.sync.dma_start(xt_t[:], xt_d[tb])
                for name, w_s, dstT in (("q", wq_s, qT), ("k", wk_s, kT)):
                    ps = psA.tile([128, TBLK], F32, tag="ps", name=f"ps_{name}_{tb}")
                    for ct in range(CT):
                        nc.tensor.matmul(
                            ps[:],
                            w_s[:, ct, :],
                            xt_t[:, ct, :],
                            start=(ct == 0),
                            stop=(ct == CT - 1),
                        )
                    nc.vector.tensor_copy(dstT[:, tb * TBLK : (tb + 1) * TBLK], ps[:])
                psv = psA.tile([128, TBLK], F32, tag="ps", name=f"ps_v_{tb}")
                for ct in range(CT):
                    nc.tensor.matmul(
                        psv[:],
                        wv_s[:, ct, :],
                        xt_t[:, ct, :],
                        start=(ct == 0),
                        stop=(ct == CT - 1),
                    )
                vt_t = vtp.tile([128, TBLK], BF16, tag="vt")
                nc.vector.tensor_copy(vt_t[:], psv[:])
                # transpose [128(d0|d1), 128k] -> [128k, (V_h0|V_h1)] on PE
                for sub in range(TBLK // 128):
                    ktg = tb * 4 + sub
                    tps = psA.tile([128, 128], BF16, tag="ps")
                    nc.tensor.transpose(
                        tps[:],
                        vt_t[:, sub * 128 : (sub + 1) * 128],
                        ident[:],
                    )
                    # strided dst: V_h0 -> slot 1, V_h1 -> slot 3
                    nc.vector.tensor_copy(
                        vN[:, ktg, 1:4:2, :],
                        tps[:].rearrange("p (a c) -> p a c", a=2),
                    )

            # out-projection for one (batch, qblock); evac="vector"|"scalar"|"mix"
            def outproj_qb(b, qb, evac="vector", chunked=False):
                t0 = b * T + qb * TBLK
                ot = outp.tile([128, CT, TBLK], BF16, tag="ot")
                for ct in range(CT):
                    opst = psA.tile([128, TBLK], F32, tag="ps")
                    nc.tensor.matmul(
                        opst[:],
                        wo_s[:, ct * 128 : (ct + 1) * 128],
                        oN[:, t0 : t0 + TBLK],
                        start=True,
                        stop=True,
                    )
                    if evac == "scalar" or (evac == "mix" and ct % 2 == 1):
                        nc.scalar.copy(ot[:, ct, :], opst[:])
                    else:
                        nc.vector.tensor_copy(ot[:, ct, :], opst[:])
                if chunked:
                    # chunked DMA so the store tail is short
                    for cc in range(0, CT, 2):
                        nc.sync.dma_start(
                            out_d[b * QB + qb, :, cc : cc + 2, :],
                            ot[:, cc : cc + 2, :],
                        )
                else:
                    nc.sync.dma_start(out_d[b * QB + qb], ot[:])

            # ---- attention for one (batch, qblock)
            def attention_qb(b, qb, outproj=True):
                t0 = b * T + qb * TBLK  # global token offset of this q block
                O_ps = psO.tile([128, HPC, TBLK], F32, tag="O", name=f"O_{b}_{qb}")
                nkt = (qb + 1) * 4
                for kt in range(nkt):
                    s = kt * 128 - qb * TBLK  # diag offset, >=0 on band
                    col0 = max(s, 0)
                    ktg = b * KT + kt
                    sT = psS.tile([128, HPC, TBLK], F32, tag="sT")
                    es = esp.tile([128, HPC, TBLK], BF16, tag="es")
                    for h in range(HPC):
                        hp = h * 64
                        nc.tensor.matmul(
                            sT[:, h, col0:TBLK],
                            kT[hp : hp + 64, b * T + kt * 128 : b * T + (kt + 1) * 128],
                            qT[hp : hp + 64, t0 + col0 : t0 + TBLK],
                            start=True,
                            stop=True,
                        )
                    nc.scalar.activation(
                        es[:, :, col0:TBLK],
                        sT[:, :, col0:TBLK],
                        mybir.ActivationFunctionType.Exp,
                        scale=0.125,
                    )
                    if s >= 0:  # diagonal tile: mask strict-lower triangle
                        nc.vector.tensor_mul(
                            es[:, :, col0 : col0 + 128],
                            es[:, :, col0 : col0 + 128],
                            mtri2[:],
                        )
                    for h in range(HPC):
                        nc.tensor.matmul(
                            O_ps[:, h, col0:TBLK],
                            vN[:, ktg, 2 * h : 2 * h + 2, :],
                            es[:, h, col0:TBLK],
                            start=(kt == 0),
                            stop=(kt == nkt - 1),
                        )
                # normalize: O rows 64:128 divided by rowsum rows 0:64 (both heads)
                rin = rinp.tile([64, HPC, TBLK], F32, tag="rin")
                nc.vector.reciprocal_approx_fast(rin[:], O_ps[0:64, :, :])
                nc.vector.tensor_mul(
                    oN[0:64, t0 : t0 + TBLK], O_ps[64:128, 0, :], rin[:, 0, :]
                )
                nc.vector.tensor_mul(
                    oN[64:128, t0 : t0 + TBLK], O_ps[64:128, 1, :], rin[:, 1, :]
                )
                if outproj:
                    outproj_qb(b, qb)

            # ---- schedule: QKV(b0) first, then b0 attention with QKV(b1)
            # blocks woven between query blocks, then b1 attention
            qkv_for_tb(0, xt_t=xt0)
            qkv_for_tb(1)
            attention_qb(0, 0)
            qkv_for_tb(2)
            attention_qb(0, 1)
            qkv_for_tb(3)
            attention_qb(0, 2)
            qkv_for_tb(4)
            attention_qb(0, 3)
            qkv_for_tb(5)
            attention_qb(1, 0)
            qkv_for_tb(6)
            attention_qb(1, 1)
            qkv_for_tb(7)
            attention_qb(1, 2)
            attention_qb(1, 3, outproj=False)
            # keep the PE clock warm through the final normalize chain so the
            # last out-projection runs at full clock
            warm2 = psA.tile([128, 128], F32, tag="ps", name="warm2")
            for _ in range(24):
                nc.tensor.matmul(warm2[:], scratch[:], scratch[:], start=True, stop=True)
            outproj_qb(1, 3, evac="mix", chunked=True)

    nc.compile()
    return nc


_NC_CACHE = None


def get_program():
    global _NC_CACHE
    if _NC_CACHE is None:
        _NC_CACHE = build_program()
    return _NC_CACHE


def make_in_maps(x, Wq, Wk, Wv, Wo):
    bf = ml_dtypes.bfloat16
    xt = np.asarray(x, np.float32).reshape(NT, C)
    # x_pre[tb, p, ct, t] = x[tb*TBLK + t, ct*128 + p]
    x_pre = np.ascontiguousarray(
        xt.reshape(NTB, TBLK, CT, 128).transpose(0, 3, 2, 1)
    ).astype(bf)
    wq_b = np.asarray(Wq, np.float32).astype(bf)
    wk_b = np.asarray(Wk, np.float32).astype(bf)
    wv_b = np.asarray(Wv, np.float32).astype(bf)
    wo_b = np.asarray(Wo, np.float32).astype(bf)
    in_maps = []
    for cid in range(N_CORES):
        sl = slice(cid * FPC, (cid + 1) * FPC)

        def wpre(w):
            # w_pre[p, ct, f] = w[ct*128 + p, f]
            return np.ascontiguousarray(
                w[:, sl].reshape(CT, 128, FPC).transpose(1, 0, 2)
            )

        in_maps.append(
            {
                "xt": x_pre,
                "wq": wpre(wq_b),
                "wk": wpre(wk_b),
                "wv": wpre(wv_b),
                "wo": np.ascontiguousarray(wo_b[sl, :]),
            }
        )
    return in_maps


def kernel(x, Wq, Wk, Wv, Wo, bo, _trace=False, _tmpdir=None):
    x = np.asarray(x, dtype=np.float32)
    in_maps = make_in_maps(x, Wq, Wk, Wv, Wo)
    nc = get_program()
    res = run_bass_kernel_spmd(
        nc, in_maps, core_ids=list(range(N_CORES)), trace=_trace, tmpdir=_tmpdir
    )
    acc = res.results[0]["outT"].astype(np.float32)
    for i in range(1, N_CORES):
        acc = acc + res.results[i]["outT"].astype(np.float32)
    # acc[tb, p, ct, t] -> out[tb*TBLK + t, ct*128 + p]
    out = np.ascontiguousarray(acc.transpose(0, 3, 2, 1)).reshape(NT, C)
    out = out + np.asarray(bo, np.float32)[None, :]
    if _trace:
        kernel._last_results = res
    return out.reshape(B, T, C).astype(np.float32)
